# revision 6
# baseline (speedup 1.0000x reference)
"""Trainium2 Bass kernel for nn_DGG_StraightThrough.

The reference's pairwise-logit MLP is mathematically dead: softmax over the
singleton feature dim is identically 1, so log_p == 0 and the gumbel logits
y equal `temp` exactly.  adj[b,i,j] = 1.0 iff temp[i,j] is among the 8
largest of row i, identical across the batch.

Sharding: row-parallel over N=2048 across 8 cores (256 rows/core, two
128-row chunks living side by side in one [128, 4096] SBUF tile).

v5 (vs the 23.9us single-queue baseline):
  - DVE keeps only the two flat MAX8s + tiny threshold negations; the
    mask compares move to the Scalar (Activation) engine as
    u8(Sign(t - thr + 1e-6)): the saturating f32->u8 cast maps sign's
    -1 to 0 and +1 to 1, and the +1e-6 (2 ulps at |thr|~2.7) keeps the
    threshold element itself at 1.  The 1e-6 window admits a false
    positive with probability ~5e-5 per core -- far inside the 2e-2
    rel-err budget.  Chunk1's compare is split DVE-half/Scalar-half to
    shrink the tail.
  - Both HWDGE queues co-stream chunk0's column split first (skewed
    toward the scalar queue, whose ring consistently starts ~2us
    earlier), then chunk1's.  All 8 cores share ~2.2TB/s of HBM, so
    only landing ORDER matters, not queue count.
  - Entry-time dma_reset/sem_clear redirected from GpSimd to Sync
    (GpSimd's multi-us launch latency); out0 leaves on the scalar queue
    overlapped with chunk1 compute, out1 on the sync queue.

Host: concatenate 8 u8 slabs, cast to f32, broadcast over B=4.
"""

import sys

import numpy as np

if "/opt/trn_rl_repo" not in sys.path:
    sys.path.insert(0, "/opt/trn_rl_repo")

B, N, K = 4, 2048, 8
N_CORES = 8
ROWS = N // N_CORES  # 256 rows per core
P = 128  # SBUF partitions
# chunk0 column split: the scalar queue's ring starts ~2us before sync's,
# so it carries the bigger share
C0B = 1280
C0A = N - C0B  # sync's share of chunk0
H = N // 2  # chunk1 split + DVE/Scalar compare split

# Hooks for a driving harness (test.py): extra kwargs for run_bass_kernel_spmd
# and the last BassKernelResults (exec_time_ns etc).
RUN_KWARGS: dict = {}
LAST_RESULT = None

_PROGRAM = None


def _build_program():
    import concourse.bass as bass
    import concourse.mybir as mybir

    class _LeanBass(bass.Bass):
        # Skip the barrier Bass.__init__ emits after const-AP registration:
        # this kernel never reads const APs, Sync's DGE table load precedes
        # its DMAs in program order, and the NRT entry pseudo-barrier already
        # orders the sem-clears.  Saves ~1us of preamble.
        _skip_init_barrier = False

        def all_engine_barrier(self, **kw):
            if _LeanBass._skip_init_barrier:
                return
            return super().all_engine_barrier(**kw)

    # Redirect the init-time sem-range drain + sem_clear from GpSimd to
    # Sync: they only need to precede the NRT pseudo-barrier, and GpSimd's
    # multi-us instruction launch latency otherwise delays its barrier
    # arrival.  One full-range drain suffices.
    def _reset_on_sync(self, semaphore_range=None):
        b = self.bass
        if getattr(b, "_lean_drained", False):
            return None
        b._lean_drained = True
        return b.sync.drain(semaphore_range=b._kernel_sem_range)

    def _clear_on_sync(self, rng):
        return self.bass.sync.sem_clear(rng)

    _LeanBass._skip_init_barrier = True
    orig_reset = bass.BassGpSimd.dma_reset
    orig_clear = bass.BassGpSimd.sem_clear
    bass.BassGpSimd.dma_reset = _reset_on_sync
    bass.BassGpSimd.sem_clear = _clear_on_sync
    try:
        nc = _LeanBass(enable_partition_id=False, monotonic_sem_count=0)
    finally:
        _LeanBass._skip_init_barrier = False
        bass.BassGpSimd.dma_reset = orig_reset
        bass.BassGpSimd.sem_clear = orig_clear
    t_in = nc.declare_dram_parameter("t", [ROWS, N], mybir.dt.float32, isOutput=False)
    # u8 wire format for the 0/1 mask (lossless); host casts back to f32
    out = nc.declare_dram_parameter("out", [ROWS, N], mybir.dt.uint8, isOutput=True)

    AF = mybir.ActivationFunctionType

    with (
        nc.sbuf_tensor([P, 2 * N], mybir.dt.float32) as tile,
        nc.sbuf_tensor([P, 2 * N], mybir.dt.uint8) as mask,
        nc.sbuf_tensor([P, 16], mybir.dt.float32) as t8,
        nc.sbuf_tensor([P, 2], mybir.dt.float32) as neg,
        # per-transfer in-DMA sems: transfers on different queues complete
        # out of order, so shared counting sems would race
        nc.semaphore("in_a0") as in_a0,
        nc.semaphore("in_b0") as in_b0,
        nc.semaphore("in_a1") as in_a1,
        nc.semaphore("in_b1") as in_b1,
        nc.semaphore("v_sem") as v_sem,
        nc.semaphore("s_sem") as s_sem,
        nc.semaphore("out_sem") as out_sem,
    ):
        # In-DMAs issued OUTSIDE the Block, right after each queue engine's
        # DGE-table preamble.  Chunk0's (skewed) column split goes first on
        # both queues, then chunk1's, so chunk0 lands early and the MAX8
        # pipeline starts as soon as possible.
        nc.sync.dma_start(out=tile[:, 0:C0A], in_=t_in[0:P, 0:C0A]).then_inc(in_a0, 16)
        nc.scalar.dma_start(out=tile[:, C0A:N], in_=t_in[0:P, C0A:N]).then_inc(
            in_b0, 16
        )
        nc.sync.dma_start(
            out=tile[:, N : N + H], in_=t_in[P : 2 * P, 0:H]
        ).then_inc(in_a1, 16)
        nc.scalar.dma_start(
            out=tile[:, N + H : 2 * N], in_=t_in[P : 2 * P, H:N]
        ).then_inc(in_b1, 16)

        # no SWDGE DMAs issued -> skip GpSimd's expensive dge_drain at exit
        with nc.Block(no_gpsimd_drain=True) as block:

            @block.vector
            def _(vector):
                # Flat MAX8 per chunk, then neg = -thr + 1e-6 for the scalar
                # engine's Sign compare.  Sem self-hops guard the same-engine
                # RAW on t8 (the scalar-ptr fetch races the in-pipeline
                # MAX8 write).  v_sem: 1=max0 2=neg0 3=max1 4=neg1 5=cmp1a
                vector.wait_ge(in_a0, 16)
                vector.wait_ge(in_b0, 16)
                vector.max(t8[:, 0:8], tile[:, 0:N]).then_inc(v_sem, 1)
                vector.wait_ge(v_sem, 1)
                vector.tensor_scalar(
                    neg[:, 0:1],
                    t8[:, 7:8],
                    -1.0,
                    1e-6,
                    mybir.AluOpType.mult,
                    mybir.AluOpType.add,
                ).then_inc(v_sem, 1)
                vector.wait_ge(in_a1, 16)
                vector.wait_ge(in_b1, 16)
                vector.max(t8[:, 8:16], tile[:, N : 2 * N]).then_inc(v_sem, 1)
                vector.wait_ge(v_sem, 3)
                vector.tensor_scalar(
                    neg[:, 1:2],
                    t8[:, 15:16],
                    -1.0,
                    1e-6,
                    mybir.AluOpType.mult,
                    mybir.AluOpType.add,
                ).then_inc(v_sem, 1)
                # chunk1 col-half a compare on DVE (exact is_ge; the scalar
                # engine handles half b in parallel)
                vector.tensor_scalar(
                    mask[:, N : N + H],
                    tile[:, N : N + H],
                    t8[:, 15:16],
                    None,
                    mybir.AluOpType.is_ge,
                ).then_inc(v_sem, 1)

            @block.scalar
            def _(scalar):
                # chunk0 compare: u8(Sign(t - thr + 1e-6)) == (t >= thr)
                # up to a ~1e-6 threshold window (negligible here); the
                # saturating cast maps -1 -> 0.
                scalar.wait_ge(v_sem, 2)
                scalar.activation(
                    mask[:, 0:N],
                    tile[:, 0:N],
                    AF.Sign,
                    bias=neg[:, 0:1],
                    scale=1.0,
                ).then_inc(s_sem, 1)
                # self-hop: the out-DMA must not read mask before the
                # activation's SBUF writes complete
                scalar.wait_ge(s_sem, 1)
                scalar.dma_start(out=out[0:P, :], in_=mask[:, 0:N]).then_inc(
                    out_sem, 16
                )
                scalar.wait_ge(v_sem, 4)
                scalar.activation(
                    mask[:, N + H : 2 * N],
                    tile[:, N + H : 2 * N],
                    AF.Sign,
                    bias=neg[:, 1:2],
                    scale=1.0,
                ).then_inc(s_sem, 1)

            @block.sync
            def _(sync):
                sync.wait_ge(v_sem, 5)
                sync.wait_ge(s_sem, 2)
                sync.dma_start(out=out[P : 2 * P, :], in_=mask[:, N : 2 * N]).then_inc(
                    out_sem, 16
                )
                sync.wait_ge(out_sem, 32)

    return nc


def kernel(**inputs: np.ndarray) -> np.ndarray:
    global _PROGRAM, LAST_RESULT
    from concourse.bass_utils import run_bass_kernel_spmd

    temp = np.ascontiguousarray(np.asarray(inputs["temp"], dtype=np.float32))
    assert temp.shape == (N, N)

    in_maps = [
        {"t": np.ascontiguousarray(temp[c * ROWS : (c + 1) * ROWS])}
        for c in range(N_CORES)
    ]

    res = None
    last_err = None
    for attempt in range(3):
        try:
            if _PROGRAM is None:
                _PROGRAM = _build_program()
            res = run_bass_kernel_spmd(
                _PROGRAM, in_maps, list(range(N_CORES)), **RUN_KWARGS
            )
            break
        except Exception as e:  # transient device wedges (e.g. NRT unrecoverable)
            last_err = e
            _PROGRAM = None
            if attempt == 2:
                raise
            import time

            time.sleep(10 * (attempt + 1))
            try:  # recreate the PJRT client, as a fresh process would
                import jax

                jax.clear_backends()
                jax.devices()
            except Exception:
                pass
    assert res is not None, last_err
    LAST_RESULT = res

    mask = np.concatenate([res.results[c]["out"] for c in range(N_CORES)], axis=0)
    mask = mask.astype(np.float32)
    return np.ascontiguousarray(np.broadcast_to(mask[None], (B, N, N)))


# revision 7
# speedup vs baseline: 1.0973x; 1.0973x over previous
"""Trainium2 Bass kernel for nn_DGG_StraightThrough.

The reference's pairwise-logit MLP is mathematically dead: softmax over the
singleton feature dim is identically 1, so log_p == 0 and the gumbel logits
y equal `temp` exactly.  adj[b,i,j] = 1.0 iff temp[i,j] is among the 8
largest of row i, identical across the batch.

Sharding: row-parallel over N=2048 across 8 cores (256 rows/core, two
128-row chunks living side by side in one [128, 4096] SBUF tile).

v6 (vs the 23.9us single-queue baseline):
  - Both HWDGE queues (sync + scalar) co-stream chunk0's column halves
    first, then chunk1's.  Hierarchical MAX8 (one per queue piece +
    16->8 merge) starts as soon as the FASTER queue's half lands -- the
    queues' ring start order flip-flops run to run, so this decouples
    DVE start from the slower one.
  - The mask compares move off DVE: chunk0 and chunk1's column half b
    run on the Scalar engine as u8(Sign(t - thr + 1e-6)) (saturating
    cast maps sign's -1 to 0; +1e-6 = 2 ulps keeps the threshold
    element at 1; the false-positive window is ~5e-5 per core, far
    inside the 2e-2 budget).  A dummy Sign at block entry prewarms the
    1.3us activation function table during the DMA stream phase.
    Chunk1's column half a uses DVE's exact is_ge in parallel.
  - Entry-time dma_reset/sem_clear redirected from GpSimd to Sync;
    both outs issue from Sync (the scalar engine must not stall its
    second Sign on a DMA issue).

Host: concatenate 8 u8 slabs, cast to f32, broadcast over B=4.
"""

import sys

import numpy as np

if "/opt/trn_rl_repo" not in sys.path:
    sys.path.insert(0, "/opt/trn_rl_repo")

B, N, K = 4, 2048, 8
N_CORES = 8
ROWS = N // N_CORES  # 256 rows per core
P = 128  # SBUF partitions
H = N // 2  # column half per queue

# Hooks for a driving harness (test.py): extra kwargs for run_bass_kernel_spmd
# and the last BassKernelResults (exec_time_ns etc).
RUN_KWARGS: dict = {}
LAST_RESULT = None

_PROGRAM = None


def _build_program():
    import concourse.bass as bass
    import concourse.mybir as mybir

    class _LeanBass(bass.Bass):
        # Skip the barrier Bass.__init__ emits after const-AP registration:
        # this kernel never reads const APs, Sync's DGE table load precedes
        # its DMAs in program order, and the NRT entry pseudo-barrier already
        # orders the sem-clears.  Saves ~1us of preamble.
        _skip_init_barrier = False

        def all_engine_barrier(self, **kw):
            if _LeanBass._skip_init_barrier:
                return
            return super().all_engine_barrier(**kw)

    # Redirect the init-time sem-range drain + sem_clear from GpSimd to
    # Sync: they only need to precede the NRT pseudo-barrier, and GpSimd's
    # multi-us instruction launch latency otherwise delays its barrier
    # arrival.  One full-range drain suffices.
    def _reset_on_sync(self, semaphore_range=None):
        b = self.bass
        if getattr(b, "_lean_drained", False):
            return None
        b._lean_drained = True
        return b.sync.drain(semaphore_range=b._kernel_sem_range)

    def _clear_on_sync(self, rng):
        return self.bass.sync.sem_clear(rng)

    _LeanBass._skip_init_barrier = True
    orig_reset = bass.BassGpSimd.dma_reset
    orig_clear = bass.BassGpSimd.sem_clear
    bass.BassGpSimd.dma_reset = _reset_on_sync
    bass.BassGpSimd.sem_clear = _clear_on_sync
    try:
        nc = _LeanBass(enable_partition_id=False, monotonic_sem_count=0)
    finally:
        _LeanBass._skip_init_barrier = False
        bass.BassGpSimd.dma_reset = orig_reset
        bass.BassGpSimd.sem_clear = orig_clear
    t_in = nc.declare_dram_parameter("t", [ROWS, N], mybir.dt.float32, isOutput=False)
    # u8 wire format for the 0/1 mask (lossless); host casts back to f32
    out = nc.declare_dram_parameter("out", [ROWS, N], mybir.dt.uint8, isOutput=True)

    AF = mybir.ActivationFunctionType

    with (
        nc.sbuf_tensor([P, 2 * N], mybir.dt.float32) as tile,
        nc.sbuf_tensor([P, 2 * N], mybir.dt.uint8) as mask,
        # per chunk: [top8 piece a | top8 piece b | merged top8]
        nc.sbuf_tensor([P, 48], mybir.dt.float32) as t8,
        nc.sbuf_tensor([P, 2], mybir.dt.float32) as neg,
        nc.sbuf_tensor([P, 4], mybir.dt.uint8) as scr8,
        # per-transfer in-DMA sems: transfers on different queues complete
        # out of order, so shared counting sems would race
        nc.semaphore("in_a0") as in_a0,
        nc.semaphore("in_b0") as in_b0,
        nc.semaphore("in_a1") as in_a1,
        nc.semaphore("in_b1") as in_b1,
        nc.semaphore("v_sem") as v_sem,
        nc.semaphore("s_sem") as s_sem,
        nc.semaphore("out_sem") as out_sem,
    ):
        # In-DMAs issued OUTSIDE the Block, right after each queue engine's
        # DGE-table preamble.  Chunk0's column halves go first on both
        # queues (4KB descriptors), then chunk1's.
        nc.sync.dma_start(out=tile[:, 0:H], in_=t_in[0:P, 0:H]).then_inc(in_a0, 16)
        nc.scalar.dma_start(out=tile[:, H:N], in_=t_in[0:P, H:N]).then_inc(in_b0, 16)
        nc.sync.dma_start(
            out=tile[:, N : N + H], in_=t_in[P : 2 * P, 0:H]
        ).then_inc(in_a1, 16)
        nc.scalar.dma_start(
            out=tile[:, N + H : 2 * N], in_=t_in[P : 2 * P, H:N]
        ).then_inc(in_b1, 16)

        # no SWDGE DMAs issued -> skip GpSimd's expensive dge_drain at exit
        with nc.Block(no_gpsimd_drain=True) as block:

            @block.vector
            def _(vector):
                # Hierarchical MAX8: one per landed queue piece, then a
                # 16->8 merge (exact: any row-top-8 element is in its
                # piece's top-8).  neg = -thr + 1e-6 feeds the scalar
                # engine's Sign compare.  Sem self-hops guard same-engine
                # RAWs on t8 (stream-read / scalar-ptr fetch race the
                # in-pipeline write of the previous op).
                # v_sem: 1=mA0 2=mB0 3=mrg0 4=neg0 5=mA1 6=mB1 7=mrg1
                #        8=neg1 9=cmp1a
                vector.wait_ge(in_a0, 16)
                vector.max(t8[:, 0:8], tile[:, 0:H]).then_inc(v_sem, 1)
                vector.wait_ge(in_b0, 16)
                vector.max(t8[:, 8:16], tile[:, H:N]).then_inc(v_sem, 1)
                vector.wait_ge(v_sem, 2)
                vector.max(t8[:, 16:24], t8[:, 0:16]).then_inc(v_sem, 1)
                vector.wait_ge(v_sem, 3)
                vector.tensor_scalar(
                    neg[:, 0:1],
                    t8[:, 23:24],
                    -1.0,
                    1e-6,
                    mybir.AluOpType.mult,
                    mybir.AluOpType.add,
                ).then_inc(v_sem, 1)
                vector.wait_ge(in_a1, 16)
                vector.max(t8[:, 24:32], tile[:, N : N + H]).then_inc(v_sem, 1)
                vector.wait_ge(in_b1, 16)
                vector.max(t8[:, 32:40], tile[:, N + H : 2 * N]).then_inc(v_sem, 1)
                vector.wait_ge(v_sem, 6)
                vector.max(t8[:, 40:48], t8[:, 24:40]).then_inc(v_sem, 1)
                vector.wait_ge(v_sem, 7)
                vector.tensor_scalar(
                    neg[:, 1:2],
                    t8[:, 47:48],
                    -1.0,
                    1e-6,
                    mybir.AluOpType.mult,
                    mybir.AluOpType.add,
                ).then_inc(v_sem, 1)
                # chunk1 col-half a compare on DVE (exact is_ge; the scalar
                # engine handles half b in parallel)
                vector.tensor_scalar(
                    mask[:, N : N + H],
                    tile[:, N : N + H],
                    t8[:, 47:48],
                    None,
                    mybir.AluOpType.is_ge,
                ).then_inc(v_sem, 1)

            @block.scalar
            def _(scalar):
                # Dummy Sign on scratch: pulls the 1.3us activation
                # function-table load into the DMA stream phase.
                scalar.activation(
                    scr8[:, 0:1],
                    neg[:, 0:1],
                    AF.Sign,
                    bias=neg[:, 1:2],
                    scale=1.0,
                )
                # chunk0 compare: u8(Sign(t - thr + 1e-6)) == (t >= thr)
                scalar.wait_ge(v_sem, 4)
                scalar.activation(
                    mask[:, 0:N],
                    tile[:, 0:N],
                    AF.Sign,
                    bias=neg[:, 0:1],
                    scale=1.0,
                ).then_inc(s_sem, 1)
                # chunk1 col-half b
                scalar.wait_ge(v_sem, 8)
                scalar.activation(
                    mask[:, N + H : 2 * N],
                    tile[:, N + H : 2 * N],
                    AF.Sign,
                    bias=neg[:, 1:2],
                    scale=1.0,
                ).then_inc(s_sem, 1)

            @block.sync
            def _(sync):
                sync.wait_ge(s_sem, 1)
                sync.dma_start(out=out[0:P, :], in_=mask[:, 0:N]).then_inc(out_sem, 16)
                sync.wait_ge(v_sem, 9)
                sync.wait_ge(s_sem, 2)
                sync.dma_start(out=out[P : 2 * P, :], in_=mask[:, N : 2 * N]).then_inc(
                    out_sem, 16
                )
                sync.wait_ge(out_sem, 32)

    return nc


def kernel(**inputs: np.ndarray) -> np.ndarray:
    global _PROGRAM, LAST_RESULT
    from concourse.bass_utils import run_bass_kernel_spmd

    temp = np.ascontiguousarray(np.asarray(inputs["temp"], dtype=np.float32))
    assert temp.shape == (N, N)

    in_maps = [
        {"t": np.ascontiguousarray(temp[c * ROWS : (c + 1) * ROWS])}
        for c in range(N_CORES)
    ]

    res = None
    last_err = None
    for attempt in range(3):
        try:
            if _PROGRAM is None:
                _PROGRAM = _build_program()
            res = run_bass_kernel_spmd(
                _PROGRAM, in_maps, list(range(N_CORES)), **RUN_KWARGS
            )
            break
        except Exception as e:  # transient device wedges (e.g. NRT unrecoverable)
            last_err = e
            _PROGRAM = None
            if attempt == 2:
                raise
            import time

            time.sleep(10 * (attempt + 1))
            try:  # recreate the PJRT client, as a fresh process would
                import jax

                jax.clear_backends()
                jax.devices()
            except Exception:
                pass
    assert res is not None, last_err
    LAST_RESULT = res

    mask = np.concatenate([res.results[c]["out"] for c in range(N_CORES)], axis=0)
    mask = mask.astype(np.float32)
    return np.ascontiguousarray(np.broadcast_to(mask[None], (B, N, N)))
